# revision 10
# baseline (speedup 1.0000x reference)
"""DenseChebConv (theta, nosum) Trainium2 kernel.

Computes out = sum_k theta_k * T_k(L) @ x @ W_k  with the Chebyshev
recurrence T_k = 2 L T_{k-1} - T_{k-2}, for B=4, N=4096, F=32, K=6, O=128.

Strategy (8 NeuronCores):
  - 2 cores per batch; core owns 2048 rows of its batch Laplacian.
  - L rows are pre-transposed + cast to fp16 on host and stay SBUF-resident,
    so HBM reads L exactly once (the recurrence re-reads it 5x from SBUF).
  - Recurrence matmuls: out[f, i] layout, 4x column-tiled PE (F=32).
  - Per-step halves exchange via pairwise AllGather (fp16, 128KB).
  - Projection accumulates over all 6 orders in PSUM via row-tiled PE.
"""

import os
import numpy as np

B, N, F, K, O = 4, 4096, 32, 6, 128
NCORES = 8
R = N // 2            # rows per core
S = R // 4            # strip width (psum free dim)
NJC = N // 128        # j-chunks (contraction)
NMC = R // 128        # natural chunks per core

_CACHE = {}


def _build(ncores=NCORES):
    import concourse.bacc as bacc
    import concourse.mybir as mybir
    import concourse.tile as tile

    dt = mybir.dt
    nc = bacc.Bacc("TRN2", target_bir_lowering=False, debug=False,
                   num_devices=ncores)

    LT_in = nc.dram_tensor("LT", [N, R], dt.float16, kind="ExternalInput")
    xhat_in = nc.dram_tensor("xhat", [128, NJC * F], dt.float16,
                             kind="ExternalInput")
    xs_in = nc.dram_tensor("xs", [128, S], dt.float32, kind="ExternalInput")
    xs16_in = nc.dram_tensor("xs16", [128, S], dt.float16,
                             kind="ExternalInput")
    wrep_in = nc.dram_tensor("wrep", [128, K * O], dt.float16,
                             kind="ExternalInput")
    id_in = nc.dram_tensor("ident", [128, 32], dt.float32,
                           kind="ExternalInput")
    y_out = nc.dram_tensor("y", [128, R], dt.float32, kind="ExternalOutput")

    RG = [[2 * i, 2 * i + 1] for i in range(ncores // 2)]
    SUB = mybir.AluOpType.subtract
    MUL = mybir.AluOpType.mult

    def tp_off(g, c):
        # transpose-psum free offset for natural chunk m = 4g + c.
        # Concurrent PE row-tiles must hit different PSUM banks: bank = g.
        return 512 * g + 32 * c

    with tile.TileContext(nc) as tc:
        with tc.tile_pool(name="big", bufs=1) as bigp, \
             tc.tile_pool(name="wts", bufs=2) as wpool, \
             tc.tile_pool(name="mst", bufs=3) as mpool, \
             tc.tile_pool(name="t16p", bufs=2) as t16p, \
             tc.tile_pool(name="twop", bufs=2) as twop, \
             tc.tile_pool(name="zps", bufs=1, space="PSUM") as zpool, \
             tc.tile_pool(name="pps", bufs=1, space="PSUM") as ppool, \
             tc.tile_pool(name="dram", bufs=1, space="DRAM") as dpool:

            # Warm the collective path: input DMA rides the gpsimd queue so
            # it is not stuck behind the big L loads on the sync queues.
            dum_sb = bigp.tile([16, 16], dt.float16, name="dum_sb")
            nc.vector.memset(dum_sb[:], 0.0)
            dummy_in = dpool.tile([16, 16], dt.float16, name="cc_dummy_in")
            dummy_out = dpool.tile([32, 16], dt.float16, name="cc_dummy_out")
            nc.gpsimd.dma_start(out=dummy_in[:], in_=dum_sb[:])
            nc.gpsimd.collective_compute(
                "AllGather", mybir.AluOpType.bypass, replica_groups=RG,
                ins=[dummy_in[:].opt()], outs=[dummy_out[:].opt()])
            # full-size warmup: the first large collective pays a one-time
            # ~8us setup cost; burn it on a dummy op during the L load.
            warm_sb = bigp.tile([128, 512], dt.float16, name="warm_sb")
            nc.vector.memset(warm_sb[:], 0.0)
            warm_in = dpool.tile([128, 512], dt.float16, name="cc_warm_in")
            warm_out = dpool.tile([256, 512], dt.float16, name="cc_warm_out")
            nc.gpsimd.dma_start(out=warm_in[:], in_=warm_sb[:])
            nc.gpsimd.collective_compute(
                "AllGather", mybir.AluOpType.bypass, replica_groups=RG,
                ins=[warm_in[:].opt()], outs=[warm_out[:].opt()])

            # Persistent SBUF
            LT = bigp.tile([128, NJC * R], dt.float16, name="LT_sb")
            WrepS = bigp.tile([128, K * O], dt.float16, name="wrep_sb")
            identS = bigp.tile([128, 32], dt.float32, name="ident_sb")
            outS = bigp.tile([128, R], dt.float32, name="out_sb")

            # Small inputs ride the scalar queue so the L stream owns sync.

            # Projection accumulator: 4 banks, lives across all 6 orders.
            ProjPs = ppool.tile([128, 4 * S], dt.float32, name="proj_ps")

            def proj(k, rhs16, start, stop):
                for g in range(4):
                    nc.tensor.matmul(
                        out=ProjPs[:, S * g:S * (g + 1)],
                        lhsT=WrepS[32 * g:32 * (g + 1), k * O:(k + 1) * O],
                        rhs=rhs16[32 * g:32 * (g + 1), :],
                        start=start, stop=stop,
                        tile_position=(32 * g, 0))

            # x-derived inputs (own/peer weight halves in separate tiles)
            TwfO = wpool.tile([128, 512], dt.float16, tag="twfo", name="twfo0")
            TwfP = wpool.tile([128, 512], dt.float16, tag="twfp", name="twfp0")
            # step-1 weights first: they gate the first matmul
            nc.scalar.dma_start(out=TwfO[:], in_=xhat_in[:, 0:512])
            nc.scalar.dma_start(out=TwfP[:], in_=xhat_in[:, 512:1024])
            Tm = {}
            Tm[0] = mpool.tile([128, S], dt.float32, tag="tm", name="tm0")
            nc.scalar.dma_start(out=Tm[0][:], in_=xs_in[:])
            T16_0 = t16p.tile([128, S], dt.float16, tag="t16")
            nc.scalar.dma_start(out=T16_0[:], in_=xs16_in[:])
            nc.scalar.dma_start(out=WrepS[:], in_=wrep_in[:])
            nc.scalar.dma_start(out=identS[:], in_=id_in[:])
            # Warm the scalar activation table early (a copy through the
            # scalar engine loads its ACT table; otherwise the first scalar
            # op at the output drain pays ~1.3us for the load).
            warm = bigp.tile([1, 2], dt.float32, name="act_warm")
            nc.scalar.copy(warm[:], identS[0:1, 0:2])

            # HAM pacing scratch: an ACT-engine copy chain paces tiny dummy
            # matmuls through each exchange window so the PE clock gate
            # (K=8/8) never sees a >3.4us idle window and the peer-half
            # matmuls after each exchange run at 2.4GHz instead of 1.2.
            # Each link writes its own tile so the dummy matmul's read
            # dependency binds to exactly that link's completion.
            ham_src = bigp.tile([128, 1024], dt.float16, name="ham_src")
            nc.vector.memset(ham_src[:], 0.0)
            NHAM = 8
            ham_tiles = []
            for i in range(NHAM):
                t = bigp.tile([128, 1024], dt.float16, name=f"ham_{i}")
                ham_tiles.append(t)
            ham_state = {"idx": 0, "links": []}

            def ham_links(n_delay, n_dummy, anchor):
                # ACT copies (~1.2us each) run back-to-back on the scalar
                # queue. The first n_delay links carry no dummy matmul (the
                # own-half matmuls already cover the window start); each of
                # the last n_dummy links releases one dummy matmul. Link 1
                # reads `anchor` so the scheduler cannot hoist the chain out
                # of its exchange window.
                links = []
                prev = None
                for i in range(n_delay + n_dummy):
                    t = ham_tiles[ham_state["idx"] % NHAM]
                    ham_state["idx"] += 1
                    if prev is None:
                        nc.scalar.copy(t[:, 0:512], anchor[:])
                    else:
                        nc.scalar.copy(t[:], prev[:])
                    prev = t
                    if i >= n_delay:
                        links.append(t)
                ham_state["links"] = links

            def ham_dummies(ztile):
                # one tiny matmul per pacing link, into the padded (unused)
                # second psum bank of the Z tile; never read back.
                for i, lk in enumerate(ham_state["links"]):
                    nc.tensor.matmul(
                        out=ztile[0:32, 512 + 64 * (i % 4):576 + 64 * (i % 4)],
                        lhsT=lk[:, 0:32], rhs=lk[:, 64:128],
                        start=True, stop=True, skip_group_check=True,
                        tile_position=(0, 0))
                ham_state["links"] = []

            # L load (SBUF-resident for the whole kernel)
            for jc in range(NJC):
                nc.sync.dma_start(
                    out=LT[:, jc * R:(jc + 1) * R],
                    in_=LT_in[128 * jc:128 * (jc + 1), :])

            proj(0, T16_0, start=True, stop=False)

            # proj for step k-1 is emitted at step k's own/peer matmul
            # boundary: the PE idles there waiting on the exchange, so the
            # projection rides for free instead of delaying the own-half
            # matmul start earlier in the queue.
            pending_proj = None

            for k in range(1, K):
                # Z[f, i] accumulation over j-chunks; 4 col-tiles = 4 strips
                Z = zpool.tile([128, 1024], dt.float32, tag="ztp", name=f"z{k}", padded_shape=[128, 2048])
                for jc in range(NJC):
                    if jc == 16:
                        if pending_proj is not None:
                            pk, pt16 = pending_proj
                            proj(pk, pt16, start=False, stop=False)
                            pending_proj = None
                        ham_dummies(Z)
                    w = TwfO if jc < 16 else TwfP
                    wof = (jc % 16) * F
                    for g in range(4):
                        nc.tensor.matmul(
                            out=Z[32 * g:32 * (g + 1), 0:S],
                            lhsT=w[:, wof:wof + F],
                            rhs=LT[:, jc * R + S * g:jc * R + S * (g + 1)],
                            start=(jc == 0), stop=(jc == NJC - 1),
                            skip_group_check=True,
                            tile_position=(0, 32 * g))

                if k < K - 1:
                    # T_k master (fp32, [f, i] strips)
                    Tm[k] = mpool.tile([128, S], dt.float32, tag="tm", name=f"tm{k}")
                    if k == 1:
                        nc.vector.tensor_copy(Tm[k][:], Z[:, 0:S])
                    else:
                        # T_k = 2 Z - T_{k-2}
                        nc.vector.scalar_tensor_tensor(
                            out=Tm[k][:], in0=Z[:, 0:S], scalar=2.0, in1=Tm[k - 2][:],
                            op0=MUL, op1=SUB)
                    # natural-layout fp16 weights for step k+1 (own half):
                    # PE transposes, one psum bank per row-group
                    TPps = zpool.tile([128, 2048], dt.float32, tag="ztp", name=f"tp{k}")
                    TwfO = wpool.tile([128, 512], dt.float16, tag="twfo", name=f"twfo{k}")
                    TwfP = wpool.tile([128, 512], dt.float16, tag="twfp", name=f"twfp{k}")
                    for g in range(4):
                        for c in range(4):
                            o = tp_off(g, c)
                            nc.tensor.transpose(
                                out=TPps[:, o:o + 32],
                                in_=Tm[k][32 * g:32 * (g + 1),
                                          128 * c:128 * (c + 1)],
                                identity=identS[32 * g:32 * (g + 1), :],
                                tile_position=(32 * g, 0))
                    # cast psum -> own weight half for step k+1
                    nc.vector.tensor_copy(
                        TwfO[:].rearrange("p (g q) -> p g q", g=4),
                        TPps[:].rearrange("p (g w) -> p g w", g=4)[:, :, 0:128])

                    # exchange halves (pairwise AllGather via DRAM bounce).
                    # The scalar queue carries ONLY the ag store per step so
                    # the store (which gates the exchange round trip) issues
                    # the moment the TwfO copy lands.
                    ag_in = dpool.tile([128, 512], dt.float16,
                                       name=f"ag_in_{k}")
                    ag_out = dpool.tile([256, 512], dt.float16,
                                        name=f"ag_out_{k}")
                    nc.scalar.dma_start(out=ag_in[:], in_=TwfO[:])
                    # pacing chain for this step's exchange window (longer
                    # for k=1: the first exchange also waits on the NRT
                    # replica barrier + collective-pipe warmup).
                    ham_links(3, 9 if k == 1 else 4, TwfO)
                    nc.gpsimd.collective_compute(
                        "AllGather", mybir.AluOpType.bypass,
                        replica_groups=RG,
                        ins=[ag_in[:].opt()], outs=[ag_out[:].opt()])

                T16_k = t16p.tile([128, S], dt.float16, tag="t16", name=f"t16_{k}")
                if k < K - 1:
                    # fp16 copy for projection: on the vector queue while an
                    # exchange is in flight (scalar stays free for the ag
                    # store).
                    nc.vector.tensor_copy(T16_k[:], Tm[k][:])
                    pending_proj = (k, T16_k)
                else:
                    # last step: no exchange and no fp32 master needed; fuse
                    # T_K-1 = 2 Z - T_K-3 straight into the fp16 projection
                    # input.
                    nc.vector.scalar_tensor_tensor(
                        out=T16_k[:], in0=Z[:, 0:S], scalar=2.0, in1=Tm[k - 2][:],
                        op0=MUL, op1=SUB)
                    proj(k, T16_k, start=False, stop=True)

                if k < K - 1:
                    # both gathered blocks -> SBUF as two contiguous loads;
                    # peer = (b0 + b1) - own (exact in fp32: fp16 addends).
                    AGsb = twop.tile([128, 1024], dt.float16, tag="agsb",
                                     name=f"agsb{k}")
                    # return-load doorbells cost ~630ns desc-gen each;
                    # issue them on separate queues so they overlap
                    # (k=1 keeps sync clear for the L-stream tail).
                    nc.gpsimd.dma_start(out=AGsb[:, 0:512],
                                        in_=ag_out[0:128, :])
                    ld2 = nc.sync if k >= 2 else nc.gpsimd
                    ld2.dma_start(out=AGsb[:, 512:1024],
                                  in_=ag_out[128:256, :])
                    SumSb = twop.tile([128, 512], dt.float32, tag="sumsb",
                                      name=f"sumsb{k}")
                    # chunked reconstruction so peer matmuls start as soon
                    # as their 128-col chunk lands.
                    for c in range(4):
                        cs = slice(128 * c, 128 * (c + 1))
                        nc.vector.tensor_add(SumSb[:, cs], AGsb[:, cs],
                                             AGsb[:, 512 + 128 * c:
                                                  512 + 128 * (c + 1)])
                        nc.vector.tensor_sub(TwfP[:, cs], SumSb[:, cs],
                                             TwfO[:, cs])

            # chunked drain: copy each psum bank out and fire its store DMA
            # immediately, alternating vector/scalar so both engines drain
            # two banks each in parallel.
            for g in range(4):
                sl = slice(S * g, S * (g + 1))
                if g % 2 == 0:
                    nc.vector.tensor_copy(outS[:, sl], ProjPs[:, sl])
                else:
                    nc.scalar.copy(outS[:, sl], ProjPs[:, sl])
                nc.sync.dma_start(out=y_out[:, sl], in_=outS[:, sl])

    nc.compile()
    return nc


def _host_prep(x, L, W, theta):
    f16, f32 = np.float16, np.float32
    th_w = theta.astype(f32)[:, None, None] * W.astype(f32)   # [K, F, O]
    wrep = np.empty((128, K * O), f16)
    for k in range(K):
        wrep[:, k * O:(k + 1) * O] = np.tile(th_w[k], (4, 1)).astype(f16)
    ident = np.tile(np.eye(32, dtype=f32), (4, 1))

    in_maps = []
    for c in range(NCORES):
        b, h = c // 2, c % 2
        rows = slice(h * R, (h + 1) * R)
        LTfull = L[b][rows, :].T          # [N, R], j global
        LT16 = np.ascontiguousarray(np.concatenate(
            [LTfull[h * R:(h + 1) * R], LTfull[(1 - h) * R:(2 - h) * R]],
            axis=0)).astype(f16)              # own j-half first
        xb = x[b]
        xloc = np.concatenate([xb[h * R:(h + 1) * R],
                               xb[(1 - h) * R:(2 - h) * R]], axis=0)
        xhat = np.ascontiguousarray(
            xloc.reshape(NJC, 128, F).transpose(1, 0, 2)).reshape(128, NJC * F)
        xs = np.ascontiguousarray(
            xb[rows].reshape(4, S, F).transpose(0, 2, 1)).reshape(128, S)
        in_maps.append({
            "LT": LT16,
            "xhat": xhat.astype(f16),
            "xs": xs.astype(f32),
            "xs16": xs.astype(f16),
            "wrep": wrep,
            "ident": ident,
        })
    return in_maps


def kernel(x, L, W, theta):
    from concourse import bass_utils

    x = np.asarray(x, dtype=np.float32)
    L = np.asarray(L, dtype=np.float32)
    W = np.asarray(W, dtype=np.float32)
    theta = np.asarray(theta, dtype=np.float32)

    if "nc" not in _CACHE:
        _CACHE["nc"] = _build()
    nc = _CACHE["nc"]

    in_maps = _host_prep(x, L, W, theta)
    trace = os.environ.get("BASS_CHEB_TRACE", "0") == "1"
    res = bass_utils.run_bass_kernel_spmd(
        nc, in_maps, core_ids=list(range(NCORES)), trace=trace)
    if trace:
        _CACHE["exec_time_ns"] = res.exec_time_ns
        _CACHE["trace"] = res.instructions_and_trace
        _CACHE["profile_json"] = res.profile_json
        print(f"HW exec time: {res.exec_time_ns} ns")

    out = np.empty((B, N, O), dtype=np.float32)
    for c in range(NCORES):
        b, h = c // 2, c % 2
        out[b, h * R:(h + 1) * R, :] = res.results[c]["y"].T
    return out



# revision 11
# speedup vs baseline: 1.0541x; 1.0541x over previous
"""DenseChebConv (theta, nosum) Trainium2 kernel.

Computes out = sum_k theta_k * T_k(L) @ x @ W_k  with the Chebyshev
recurrence T_k = 2 L T_{k-1} - T_{k-2}, for B=4, N=4096, F=32, K=6, O=128.

Strategy (8 NeuronCores):
  - 2 cores per batch; core owns 2048 rows of its batch Laplacian.
  - L rows are pre-transposed + cast to fp16 on host and stay SBUF-resident,
    so HBM reads L exactly once (the recurrence re-reads it 5x from SBUF).
  - Recurrence matmuls: out[f, i] layout, 4x column-tiled PE (F=32).
  - Per-step halves exchange via pairwise AllGather (fp16, 128KB).
  - Projection accumulates over all 6 orders in PSUM via row-tiled PE.
"""

import os
import numpy as np

B, N, F, K, O = 4, 4096, 32, 6, 128
NCORES = 8
R = N // 2            # rows per core
S = R // 4            # strip width (psum free dim)
NJC = N // 128        # j-chunks (contraction)
NMC = R // 128        # natural chunks per core

_CACHE = {}


def _build(ncores=NCORES):
    import concourse.bacc as bacc
    import concourse.mybir as mybir
    import concourse.tile as tile

    dt = mybir.dt
    nc = bacc.Bacc("TRN2", target_bir_lowering=False, debug=False,
                   num_devices=ncores)

    LT_in = nc.dram_tensor("LT", [N, R], dt.float16, kind="ExternalInput")
    xhat_in = nc.dram_tensor("xhat", [128, NJC * F], dt.float16,
                             kind="ExternalInput")
    xs_in = nc.dram_tensor("xs", [128, S], dt.float32, kind="ExternalInput")
    xs16_in = nc.dram_tensor("xs16", [128, S], dt.float16,
                             kind="ExternalInput")
    wrep_in = nc.dram_tensor("wrep", [128, K * O], dt.float16,
                             kind="ExternalInput")
    id_in = nc.dram_tensor("ident", [128, 32], dt.float32,
                           kind="ExternalInput")
    y_out = nc.dram_tensor("y", [128, R], dt.float32, kind="ExternalOutput")

    RG = [[2 * i, 2 * i + 1] for i in range(ncores // 2)]
    SUB = mybir.AluOpType.subtract
    MUL = mybir.AluOpType.mult

    def tp_off(g, c):
        # transpose-psum free offset for natural chunk m = 4g + c.
        # Concurrent PE row-tiles must hit different PSUM banks: bank = g.
        return 512 * g + 32 * c

    with tile.TileContext(nc) as tc:
        with tc.tile_pool(name="big", bufs=1) as bigp, \
             tc.tile_pool(name="wts", bufs=2) as wpool, \
             tc.tile_pool(name="mst", bufs=3) as mpool, \
             tc.tile_pool(name="t16p", bufs=2) as t16p, \
             tc.tile_pool(name="twop", bufs=2) as twop, \
             tc.tile_pool(name="zps", bufs=1, space="PSUM") as zpool, \
             tc.tile_pool(name="pps", bufs=1, space="PSUM") as ppool, \
             tc.tile_pool(name="dram", bufs=1, space="DRAM") as dpool:

            # Warm the collective path: input DMA rides the gpsimd queue so
            # it is not stuck behind the big L loads on the sync queues.
            dum_sb = bigp.tile([16, 16], dt.float16, name="dum_sb")
            nc.vector.memset(dum_sb[:], 0.0)
            dummy_in = dpool.tile([16, 16], dt.float16, name="cc_dummy_in")
            dummy_out = dpool.tile([32, 16], dt.float16, name="cc_dummy_out")
            nc.gpsimd.dma_start(out=dummy_in[:], in_=dum_sb[:])
            nc.gpsimd.collective_compute(
                "AllGather", mybir.AluOpType.bypass, replica_groups=RG,
                ins=[dummy_in[:].opt()], outs=[dummy_out[:].opt()])

            # Persistent SBUF
            LT = bigp.tile([128, NJC * R], dt.float16, name="LT_sb")
            WrepS = bigp.tile([128, K * O], dt.float16, name="wrep_sb")
            identS = bigp.tile([128, 32], dt.float32, name="ident_sb")
            outS = bigp.tile([128, R], dt.float32, name="out_sb")

            # Small inputs ride the scalar queue so the L stream owns sync.

            # Projection accumulator: 4 banks, lives across all 6 orders.
            ProjPs = ppool.tile([128, 4 * S], dt.float32, name="proj_ps")

            def proj(k, rhs16, start, stop):
                for g in range(4):
                    nc.tensor.matmul(
                        out=ProjPs[:, S * g:S * (g + 1)],
                        lhsT=WrepS[32 * g:32 * (g + 1), k * O:(k + 1) * O],
                        rhs=rhs16[32 * g:32 * (g + 1), :],
                        start=start, stop=stop,
                        tile_position=(32 * g, 0))

            # x-derived inputs (own/peer weight halves in separate tiles)
            TwfO = wpool.tile([128, 512], dt.float16, tag="twfo", name="twfo0")
            TwfP = wpool.tile([128, 512], dt.float16, tag="twfp", name="twfp0")
            # step-1 weights first: they gate the first matmul
            nc.scalar.dma_start(out=TwfO[:], in_=xhat_in[:, 0:512])
            nc.scalar.dma_start(out=TwfP[:], in_=xhat_in[:, 512:1024])
            Tm = {}
            Tm[0] = mpool.tile([128, S], dt.float32, tag="tm", name="tm0")
            nc.scalar.dma_start(out=Tm[0][:], in_=xs_in[:])
            T16_0 = t16p.tile([128, S], dt.float16, tag="t16")
            nc.scalar.dma_start(out=T16_0[:], in_=xs16_in[:])
            nc.scalar.dma_start(out=WrepS[:], in_=wrep_in[:])
            nc.scalar.dma_start(out=identS[:], in_=id_in[:])
            # Warm the scalar activation table early (a copy through the
            # scalar engine loads its ACT table; otherwise the first scalar
            # op at the output drain pays ~1.3us for the load).
            warm = bigp.tile([1, 2], dt.float32, name="act_warm")
            nc.scalar.copy(warm[:], identS[0:1, 0:2])

            # HAM pacing scratch: an ACT-engine copy chain paces tiny dummy
            # matmuls through each exchange window so the PE clock gate
            # (K=8/8) never sees a >3.4us idle window and the peer-half
            # matmuls after each exchange run at 2.4GHz instead of 1.2.
            # Each link writes its own tile so the dummy matmul's read
            # dependency binds to exactly that link's completion.
            ham_src = bigp.tile([128, 1024], dt.float16, name="ham_src")
            nc.vector.memset(ham_src[:], 0.0)
            NHAM = 8
            ham_tiles = []
            for i in range(NHAM):
                t = bigp.tile([128, 1024], dt.float16, name=f"ham_{i}")
                ham_tiles.append(t)
            ham_state = {"idx": 0, "links": []}

            def ham_links(n_delay, n_dummy, anchor):
                # ACT copies (~1.2us each) run back-to-back on the scalar
                # queue. The first n_delay links carry no dummy matmul (the
                # own-half matmuls already cover the window start); each of
                # the last n_dummy links releases one dummy matmul. Link 1
                # reads `anchor` so the scheduler cannot hoist the chain out
                # of its exchange window.
                links = []
                prev = None
                for i in range(n_delay + n_dummy):
                    t = ham_tiles[ham_state["idx"] % NHAM]
                    ham_state["idx"] += 1
                    if prev is None:
                        nc.scalar.copy(t[:, 0:512], anchor[:])
                    else:
                        nc.scalar.copy(t[:], prev[:])
                    prev = t
                    if i >= n_delay:
                        links.append(t)
                ham_state["links"] = links

            def ham_dummies(ztile):
                # one tiny matmul per pacing link, into the padded (unused)
                # second psum bank of the Z tile; never read back.
                for i, lk in enumerate(ham_state["links"]):
                    nc.tensor.matmul(
                        out=ztile[0:32, 512 + 64 * (i % 4):576 + 64 * (i % 4)],
                        lhsT=lk[:, 0:32], rhs=lk[:, 64:128],
                        start=True, stop=True, skip_group_check=True,
                        tile_position=(0, 0))
                ham_state["links"] = []

            # L load (SBUF-resident for the whole kernel)
            for jc in range(NJC):
                nc.sync.dma_start(
                    out=LT[:, jc * R:(jc + 1) * R],
                    in_=LT_in[128 * jc:128 * (jc + 1), :])

            proj(0, T16_0, start=True, stop=False)

            # proj for step k-1 is emitted at step k's own/peer matmul
            # boundary: the PE idles there waiting on the exchange, so the
            # projection rides for free instead of delaying the own-half
            # matmul start earlier in the queue.
            pending_proj = None

            for k in range(1, K):
                # Z[f, i] accumulation over j-chunks; 4 col-tiles = 4 strips
                Z = zpool.tile([128, 1024], dt.float32, tag="ztp", name=f"z{k}", padded_shape=[128, 2048])
                for jc in range(NJC):
                    if jc == 16:
                        if pending_proj is not None:
                            pk, pt16 = pending_proj
                            proj(pk, pt16, start=False, stop=False)
                            pending_proj = None
                        ham_dummies(Z)
                    w = TwfO if jc < 16 else TwfP
                    wof = (jc % 16) * F
                    for g in range(4):
                        nc.tensor.matmul(
                            out=Z[32 * g:32 * (g + 1), 0:S],
                            lhsT=w[:, wof:wof + F],
                            rhs=LT[:, jc * R + S * g:jc * R + S * (g + 1)],
                            start=(jc == 0), stop=(jc == NJC - 1),
                            skip_group_check=True,
                            tile_position=(0, 32 * g))

                if k < K - 1:
                    # T_k master (fp32, [f, i] strips)
                    Tm[k] = mpool.tile([128, S], dt.float32, tag="tm", name=f"tm{k}")
                    if k == 1:
                        nc.vector.tensor_copy(Tm[k][:], Z[:, 0:S])
                    else:
                        # T_k = 2 Z - T_{k-2}
                        nc.vector.scalar_tensor_tensor(
                            out=Tm[k][:], in0=Z[:, 0:S], scalar=2.0, in1=Tm[k - 2][:],
                            op0=MUL, op1=SUB)
                    # natural-layout fp16 weights for step k+1 (own half):
                    # PE transposes, one psum bank per row-group
                    TPps = zpool.tile([128, 2048], dt.float32, tag="ztp", name=f"tp{k}")
                    TwfO = wpool.tile([128, 512], dt.float16, tag="twfo", name=f"twfo{k}")
                    TwfP = wpool.tile([128, 512], dt.float16, tag="twfp", name=f"twfp{k}")
                    for g in range(4):
                        for c in range(4):
                            o = tp_off(g, c)
                            nc.tensor.transpose(
                                out=TPps[:, o:o + 32],
                                in_=Tm[k][32 * g:32 * (g + 1),
                                          128 * c:128 * (c + 1)],
                                identity=identS[32 * g:32 * (g + 1), :],
                                tile_position=(32 * g, 0))
                    # cast psum -> own weight half for step k+1
                    nc.vector.tensor_copy(
                        TwfO[:].rearrange("p (g q) -> p g q", g=4),
                        TPps[:].rearrange("p (g w) -> p g w", g=4)[:, :, 0:128])

                    # exchange halves (pairwise AllGather via DRAM bounce).
                    # The scalar queue carries ONLY the ag store per step so
                    # the store (which gates the exchange round trip) issues
                    # the moment the TwfO copy lands.
                    ag_in = dpool.tile([128, 512], dt.float16,
                                       name=f"ag_in_{k}")
                    ag_out = dpool.tile([256, 512], dt.float16,
                                        name=f"ag_out_{k}")
                    nc.scalar.dma_start(out=ag_in[:], in_=TwfO[:])
                    # pacing chain for this step's exchange window (longer
                    # for k=1: the first exchange also waits on the NRT
                    # replica barrier + collective-pipe warmup).
                    ham_links(3, 9 if k == 1 else 4, TwfO)
                    nc.gpsimd.collective_compute(
                        "AllGather", mybir.AluOpType.bypass,
                        replica_groups=RG,
                        ins=[ag_in[:].opt()], outs=[ag_out[:].opt()])

                T16_k = t16p.tile([128, S], dt.float16, tag="t16", name=f"t16_{k}")
                if k < K - 1:
                    # fp16 copy for projection: on the vector queue while an
                    # exchange is in flight (scalar stays free for the ag
                    # store).
                    nc.vector.tensor_copy(T16_k[:], Tm[k][:])
                    pending_proj = (k, T16_k)
                else:
                    # last step: no exchange and no fp32 master needed; fuse
                    # T_K-1 = 2 Z - T_K-3 straight into the fp16 projection
                    # input.
                    nc.vector.scalar_tensor_tensor(
                        out=T16_k[:], in0=Z[:, 0:S], scalar=2.0, in1=Tm[k - 2][:],
                        op0=MUL, op1=SUB)
                    proj(k, T16_k, start=False, stop=True)

                if k < K - 1:
                    # both gathered blocks -> SBUF as two contiguous loads;
                    # peer = (b0 + b1) - own (exact in fp32: fp16 addends).
                    AGsb = twop.tile([128, 1024], dt.float16, tag="agsb",
                                     name=f"agsb{k}")
                    # return-load doorbells cost ~630ns desc-gen each;
                    # issue them on separate queues so they overlap
                    # (k=1 keeps sync clear for the L-stream tail).
                    nc.gpsimd.dma_start(out=AGsb[:, 0:512],
                                        in_=ag_out[0:128, :])
                    ld2 = nc.sync if k >= 2 else nc.gpsimd
                    ld2.dma_start(out=AGsb[:, 512:1024],
                                  in_=ag_out[128:256, :])
                    SumSb = twop.tile([128, 512], dt.float32, tag="sumsb",
                                      name=f"sumsb{k}")
                    # chunked reconstruction so peer matmuls start as soon
                    # as their 128-col chunk lands.
                    for c in range(4):
                        cs = slice(128 * c, 128 * (c + 1))
                        nc.vector.tensor_add(SumSb[:, cs], AGsb[:, cs],
                                             AGsb[:, 512 + 128 * c:
                                                  512 + 128 * (c + 1)])
                        nc.vector.tensor_sub(TwfP[:, cs], SumSb[:, cs],
                                             TwfO[:, cs])

            # chunked drain: copy each psum bank out and fire its store DMA
            # immediately, alternating vector/scalar so both engines drain
            # two banks each in parallel.
            for g in range(4):
                sl = slice(S * g, S * (g + 1))
                if g % 2 == 0:
                    nc.vector.tensor_copy(outS[:, sl], ProjPs[:, sl])
                else:
                    nc.scalar.copy(outS[:, sl], ProjPs[:, sl])
                nc.sync.dma_start(out=y_out[:, sl], in_=outS[:, sl])

    nc.compile()
    return nc


def _host_prep(x, L, W, theta):
    f16, f32 = np.float16, np.float32
    th_w = theta.astype(f32)[:, None, None] * W.astype(f32)   # [K, F, O]
    wrep = np.empty((128, K * O), f16)
    for k in range(K):
        wrep[:, k * O:(k + 1) * O] = np.tile(th_w[k], (4, 1)).astype(f16)
    ident = np.tile(np.eye(32, dtype=f32), (4, 1))

    in_maps = []
    for c in range(NCORES):
        b, h = c // 2, c % 2
        rows = slice(h * R, (h + 1) * R)
        LTfull = L[b][rows, :].T          # [N, R], j global
        LT16 = np.ascontiguousarray(np.concatenate(
            [LTfull[h * R:(h + 1) * R], LTfull[(1 - h) * R:(2 - h) * R]],
            axis=0)).astype(f16)              # own j-half first
        xb = x[b]
        xloc = np.concatenate([xb[h * R:(h + 1) * R],
                               xb[(1 - h) * R:(2 - h) * R]], axis=0)
        xhat = np.ascontiguousarray(
            xloc.reshape(NJC, 128, F).transpose(1, 0, 2)).reshape(128, NJC * F)
        xs = np.ascontiguousarray(
            xb[rows].reshape(4, S, F).transpose(0, 2, 1)).reshape(128, S)
        in_maps.append({
            "LT": LT16,
            "xhat": xhat.astype(f16),
            "xs": xs.astype(f32),
            "xs16": xs.astype(f16),
            "wrep": wrep,
            "ident": ident,
        })
    return in_maps


def kernel(x, L, W, theta):
    from concourse import bass_utils

    x = np.asarray(x, dtype=np.float32)
    L = np.asarray(L, dtype=np.float32)
    W = np.asarray(W, dtype=np.float32)
    theta = np.asarray(theta, dtype=np.float32)

    if "nc" not in _CACHE:
        _CACHE["nc"] = _build()
    nc = _CACHE["nc"]

    in_maps = _host_prep(x, L, W, theta)
    trace = os.environ.get("BASS_CHEB_TRACE", "0") == "1"
    res = bass_utils.run_bass_kernel_spmd(
        nc, in_maps, core_ids=list(range(NCORES)), trace=trace)
    if trace:
        _CACHE["exec_time_ns"] = res.exec_time_ns
        _CACHE["trace"] = res.instructions_and_trace
        _CACHE["profile_json"] = res.profile_json
        print(f"HW exec time: {res.exec_time_ns} ns")

    out = np.empty((B, N, O), dtype=np.float32)
    for c in range(NCORES):
        b, h = c // 2, c % 2
        out[b, h * R:(h + 1) * R, :] = res.results[c]["y"].T
    return out



# revision 12
# speedup vs baseline: 1.0804x; 1.0250x over previous
"""DenseChebConv (theta, nosum) Trainium2 kernel.

Computes out = sum_k theta_k * T_k(L) @ x @ W_k  with the Chebyshev
recurrence T_k = 2 L T_{k-1} - T_{k-2}, for B=4, N=4096, F=32, K=6, O=128.

Strategy (8 NeuronCores):
  - 2 cores per batch; core owns 2048 rows of its batch Laplacian.
  - L rows are pre-transposed + cast to fp16 on host and stay SBUF-resident,
    so HBM reads L exactly once (the recurrence re-reads it 5x from SBUF).
  - Recurrence matmuls: out[f, i] layout, 4x column-tiled PE (F=32).
  - Per-step halves exchange via pairwise AllGather (fp16, 128KB).
  - Projection accumulates over all 6 orders in PSUM via row-tiled PE.
"""

import os
import numpy as np

B, N, F, K, O = 4, 4096, 32, 6, 128
NCORES = 8
R = N // 2            # rows per core
S = R // 4            # strip width (psum free dim)
NJC = N // 128        # j-chunks (contraction)
NMC = R // 128        # natural chunks per core

_CACHE = {}


def _build(ncores=NCORES):
    import concourse.bacc as bacc
    import concourse.mybir as mybir
    import concourse.tile as tile

    dt = mybir.dt
    nc = bacc.Bacc("TRN2", target_bir_lowering=False, debug=False,
                   num_devices=ncores)

    LT_in = nc.dram_tensor("LT", [N, R], dt.float16, kind="ExternalInput")
    xhat_in = nc.dram_tensor("xhat", [128, NJC * F], dt.float16,
                             kind="ExternalInput")
    xs_in = nc.dram_tensor("xs", [128, S], dt.float32, kind="ExternalInput")
    xs16_in = nc.dram_tensor("xs16", [128, S], dt.float16,
                             kind="ExternalInput")
    wrep_in = nc.dram_tensor("wrep", [128, K * O], dt.float16,
                             kind="ExternalInput")
    id_in = nc.dram_tensor("ident", [128, 32], dt.float32,
                           kind="ExternalInput")
    y_out = nc.dram_tensor("y", [128, R], dt.float32, kind="ExternalOutput")

    RG = [[2 * i, 2 * i + 1] for i in range(ncores // 2)]
    SUB = mybir.AluOpType.subtract
    MUL = mybir.AluOpType.mult

    def tp_off(g, c):
        # transpose-psum free offset for natural chunk m = 4g + c.
        # Concurrent PE row-tiles must hit different PSUM banks: bank = g.
        return 512 * g + 32 * c

    with tile.TileContext(nc) as tc:
        with tc.tile_pool(name="big", bufs=1) as bigp, \
             tc.tile_pool(name="wts", bufs=2) as wpool, \
             tc.tile_pool(name="mst", bufs=3) as mpool, \
             tc.tile_pool(name="t16p", bufs=2) as t16p, \
             tc.tile_pool(name="twop", bufs=2) as twop, \
             tc.tile_pool(name="zps", bufs=1, space="PSUM") as zpool, \
             tc.tile_pool(name="pps", bufs=1, space="PSUM") as ppool, \
             tc.tile_pool(name="dram", bufs=1, space="DRAM") as dpool:

            # Warm the collective path: input DMA rides the gpsimd queue so
            # it is not stuck behind the big L loads on the sync queues.
            dum_sb = bigp.tile([16, 16], dt.float16, name="dum_sb")
            nc.vector.memset(dum_sb[:], 0.0)
            dummy_in = dpool.tile([16, 16], dt.float16, name="cc_dummy_in")
            dummy_out = dpool.tile([32, 16], dt.float16, name="cc_dummy_out")
            nc.gpsimd.dma_start(out=dummy_in[:], in_=dum_sb[:])
            nc.gpsimd.collective_compute(
                "AllGather", mybir.AluOpType.bypass, replica_groups=RG,
                ins=[dummy_in[:].opt()], outs=[dummy_out[:].opt()])
            # full-size warmup: keeps the collective stream hot through the
            # L-load window so the first real exchange starts promptly.
            warm_sb = bigp.tile([128, 512], dt.float16, name="warm_sb")
            nc.vector.memset(warm_sb[:], 0.0)
            warm_in = dpool.tile([128, 512], dt.float16, name="cc_warm_in")
            warm_out = dpool.tile([256, 512], dt.float16, name="cc_warm_out")
            nc.gpsimd.dma_start(out=warm_in[:], in_=warm_sb[:])
            nc.gpsimd.collective_compute(
                "AllGather", mybir.AluOpType.bypass, replica_groups=RG,
                ins=[warm_in[:].opt()], outs=[warm_out[:].opt()])

            # Persistent SBUF
            LT = bigp.tile([128, NJC * R], dt.float16, name="LT_sb")
            WrepS = bigp.tile([128, K * O], dt.float16, name="wrep_sb")
            identS = bigp.tile([128, 32], dt.float32, name="ident_sb")
            outS = bigp.tile([128, R], dt.float32, name="out_sb")

            # Small inputs ride the scalar queue so the L stream owns sync.

            # Projection accumulator: 4 banks, lives across all 6 orders.
            ProjPs = ppool.tile([128, 4 * S], dt.float32, name="proj_ps")

            def proj(k, rhs16, start, stop):
                for g in range(4):
                    nc.tensor.matmul(
                        out=ProjPs[:, S * g:S * (g + 1)],
                        lhsT=WrepS[32 * g:32 * (g + 1), k * O:(k + 1) * O],
                        rhs=rhs16[32 * g:32 * (g + 1), :],
                        start=start, stop=stop,
                        tile_position=(32 * g, 0))

            # x-derived inputs (own/peer weight halves in separate tiles)
            TwfO = wpool.tile([128, 512], dt.float16, tag="twfo", name="twfo0")
            TwfP = wpool.tile([128, 512], dt.float16, tag="twfp", name="twfp0")
            # step-1 weights first: they gate the first matmul
            nc.scalar.dma_start(out=TwfO[:], in_=xhat_in[:, 0:512])
            nc.scalar.dma_start(out=TwfP[:], in_=xhat_in[:, 512:1024])
            Tm = {}
            Tm[0] = mpool.tile([128, S], dt.float32, tag="tm", name="tm0")
            nc.scalar.dma_start(out=Tm[0][:], in_=xs_in[:])
            T16_0 = t16p.tile([128, S], dt.float16, tag="t16")
            nc.scalar.dma_start(out=T16_0[:], in_=xs16_in[:])
            nc.scalar.dma_start(out=WrepS[:], in_=wrep_in[:])
            nc.scalar.dma_start(out=identS[:], in_=id_in[:])
            # Warm the scalar activation table early (a copy through the
            # scalar engine loads its ACT table; otherwise the first scalar
            # op at the output drain pays ~1.3us for the load).
            warm = bigp.tile([1, 2], dt.float32, name="act_warm")
            nc.scalar.copy(warm[:], identS[0:1, 0:2])

            # HAM pacing scratch: an ACT-engine copy chain paces tiny dummy
            # matmuls through each exchange window so the PE clock gate
            # (K=8/8) never sees a >3.4us idle window and the peer-half
            # matmuls after each exchange run at 2.4GHz instead of 1.2.
            # Each link writes its own tile so the dummy matmul's read
            # dependency binds to exactly that link's completion.
            ham_src = bigp.tile([128, 1024], dt.float16, name="ham_src")
            nc.vector.memset(ham_src[:], 0.0)
            NHAM = 8
            ham_tiles = []
            for i in range(NHAM):
                t = bigp.tile([128, 1024], dt.float16, name=f"ham_{i}")
                ham_tiles.append(t)
            ham_state = {"idx": 0, "links": []}

            def ham_links(n_delay, n_dummy, anchor):
                # ACT copies (~1.2us each) run back-to-back on the scalar
                # queue. The first n_delay links carry no dummy matmul (the
                # own-half matmuls already cover the window start); each of
                # the last n_dummy links releases one dummy matmul. Link 1
                # reads `anchor` so the scheduler cannot hoist the chain out
                # of its exchange window.
                links = []
                prev = None
                for i in range(n_delay + n_dummy):
                    t = ham_tiles[ham_state["idx"] % NHAM]
                    ham_state["idx"] += 1
                    if prev is None:
                        nc.scalar.copy(t[:, 0:512], anchor[:])
                    else:
                        nc.scalar.copy(t[:], prev[:])
                    prev = t
                    if i >= n_delay:
                        links.append(t)
                ham_state["links"] = links

            def ham_dummies(ztile):
                # one tiny matmul per pacing link, into the padded (unused)
                # second psum bank of the Z tile; never read back.
                for i, lk in enumerate(ham_state["links"]):
                    nc.tensor.matmul(
                        out=ztile[0:32, 512 + 64 * (i % 4):576 + 64 * (i % 4)],
                        lhsT=lk[:, 0:32], rhs=lk[:, 64:128],
                        start=True, stop=True, skip_group_check=True,
                        tile_position=(0, 0))
                ham_state["links"] = []

            # L load (SBUF-resident for the whole kernel)
            for jc in range(NJC):
                nc.sync.dma_start(
                    out=LT[:, jc * R:(jc + 1) * R],
                    in_=LT_in[128 * jc:128 * (jc + 1), :])

            proj(0, T16_0, start=True, stop=False)

            # proj for step k-1 is emitted at step k's own/peer matmul
            # boundary: the PE idles there waiting on the exchange, so the
            # projection rides for free instead of delaying the own-half
            # matmul start earlier in the queue.
            pending_proj = None

            for k in range(1, K):
                # Z[f, i] accumulation over j-chunks; 4 col-tiles = 4 strips
                Z = zpool.tile([128, 1024], dt.float32, tag="ztp", name=f"z{k}", padded_shape=[128, 2048])
                for jc in range(NJC):
                    if jc == 16:
                        if pending_proj is not None:
                            pk, pt16 = pending_proj
                            proj(pk, pt16, start=False, stop=False)
                            pending_proj = None
                        ham_dummies(Z)
                    w = TwfO if jc < 16 else TwfP
                    wof = (jc % 16) * F
                    for g in range(4):
                        nc.tensor.matmul(
                            out=Z[32 * g:32 * (g + 1), 0:S],
                            lhsT=w[:, wof:wof + F],
                            rhs=LT[:, jc * R + S * g:jc * R + S * (g + 1)],
                            start=(jc == 0), stop=(jc == NJC - 1),
                            skip_group_check=True,
                            tile_position=(0, 32 * g))

                if k < K - 1:
                    # T_k master (fp32, [f, i] strips)
                    Tm[k] = mpool.tile([128, S], dt.float32, tag="tm", name=f"tm{k}")
                    if k == 1:
                        nc.vector.tensor_copy(Tm[k][:], Z[:, 0:S])
                    else:
                        # T_k = 2 Z - T_{k-2}
                        nc.vector.scalar_tensor_tensor(
                            out=Tm[k][:], in0=Z[:, 0:S], scalar=2.0, in1=Tm[k - 2][:],
                            op0=MUL, op1=SUB)
                    # natural-layout fp16 weights for step k+1 (own half):
                    # PE transposes, one psum bank per row-group
                    TPps = zpool.tile([128, 2048], dt.float32, tag="ztp", name=f"tp{k}")
                    TwfO = wpool.tile([128, 512], dt.float16, tag="twfo", name=f"twfo{k}")
                    TwfP = wpool.tile([128, 512], dt.float16, tag="twfp", name=f"twfp{k}")
                    for g in range(4):
                        for c in range(4):
                            o = tp_off(g, c)
                            nc.tensor.transpose(
                                out=TPps[:, o:o + 32],
                                in_=Tm[k][32 * g:32 * (g + 1),
                                          128 * c:128 * (c + 1)],
                                identity=identS[32 * g:32 * (g + 1), :],
                                tile_position=(32 * g, 0))
                    # cast psum -> own weight half for step k+1
                    nc.vector.tensor_copy(
                        TwfO[:].rearrange("p (g q) -> p g q", g=4),
                        TPps[:].rearrange("p (g w) -> p g w", g=4)[:, :, 0:128])

                    # exchange halves (pairwise AllGather via DRAM bounce).
                    # The scalar queue carries ONLY the ag store per step so
                    # the store (which gates the exchange round trip) issues
                    # the moment the TwfO copy lands.
                    ag_in = dpool.tile([128, 512], dt.float16,
                                       name=f"ag_in_{k}")
                    ag_out = dpool.tile([256, 512], dt.float16,
                                        name=f"ag_out_{k}")
                    nc.scalar.dma_start(out=ag_in[:], in_=TwfO[:])
                    # pacing chain for this step's exchange window (longer
                    # for k=1: the first exchange also waits on the NRT
                    # replica barrier + collective-pipe warmup).
                    ham_links(3, 9 if k == 1 else 4, TwfO)
                    nc.gpsimd.collective_compute(
                        "AllGather", mybir.AluOpType.bypass,
                        replica_groups=RG,
                        ins=[ag_in[:].opt()], outs=[ag_out[:].opt()])

                T16_k = t16p.tile([128, S], dt.float16, tag="t16", name=f"t16_{k}")
                if k < K - 1:
                    # fp16 copy for projection: on the vector queue while an
                    # exchange is in flight (scalar stays free for the ag
                    # store).
                    nc.vector.tensor_copy(T16_k[:], Tm[k][:])
                    pending_proj = (k, T16_k)
                else:
                    # last step: no exchange and no fp32 master needed; fuse
                    # T_K-1 = 2 Z - T_K-3 straight into the fp16 projection
                    # input.
                    nc.vector.scalar_tensor_tensor(
                        out=T16_k[:], in0=Z[:, 0:S], scalar=2.0, in1=Tm[k - 2][:],
                        op0=MUL, op1=SUB)
                    proj(k, T16_k, start=False, stop=True)

                if k < K - 1:
                    # both gathered blocks -> SBUF as two contiguous loads;
                    # peer = (b0 + b1) - own (exact in fp32: fp16 addends).
                    AGsb = twop.tile([128, 1024], dt.float16, tag="agsb",
                                     name=f"agsb{k}")
                    # return-load doorbells cost ~630ns desc-gen each;
                    # issue them on separate queues so they overlap
                    # (k=1 keeps sync clear for the L-stream tail).
                    nc.gpsimd.dma_start(out=AGsb[:, 0:512],
                                        in_=ag_out[0:128, :])
                    ld2 = nc.sync if k >= 2 else nc.gpsimd
                    ld2.dma_start(out=AGsb[:, 512:1024],
                                  in_=ag_out[128:256, :])
                    SumSb = twop.tile([128, 512], dt.float32, tag="sumsb",
                                      name=f"sumsb{k}")
                    # chunked reconstruction so peer matmuls start as soon
                    # as their 128-col chunk lands.
                    for c in range(4):
                        cs = slice(128 * c, 128 * (c + 1))
                        nc.vector.tensor_add(SumSb[:, cs], AGsb[:, cs],
                                             AGsb[:, 512 + 128 * c:
                                                  512 + 128 * (c + 1)])
                        nc.vector.tensor_sub(TwfP[:, cs], SumSb[:, cs],
                                             TwfO[:, cs])

            # chunked drain: copy each psum bank out and fire its store DMA
            # immediately, alternating vector/scalar so both engines drain
            # two banks each in parallel.
            for g in range(4):
                sl = slice(S * g, S * (g + 1))
                if g % 2 == 0:
                    nc.vector.tensor_copy(outS[:, sl], ProjPs[:, sl])
                else:
                    nc.scalar.copy(outS[:, sl], ProjPs[:, sl])
                nc.sync.dma_start(out=y_out[:, sl], in_=outS[:, sl])

    nc.compile()
    return nc


def _host_prep(x, L, W, theta):
    f16, f32 = np.float16, np.float32
    th_w = theta.astype(f32)[:, None, None] * W.astype(f32)   # [K, F, O]
    wrep = np.empty((128, K * O), f16)
    for k in range(K):
        wrep[:, k * O:(k + 1) * O] = np.tile(th_w[k], (4, 1)).astype(f16)
    ident = np.tile(np.eye(32, dtype=f32), (4, 1))

    in_maps = []
    for c in range(NCORES):
        b, h = c // 2, c % 2
        rows = slice(h * R, (h + 1) * R)
        LTfull = L[b][rows, :].T          # [N, R], j global
        LT16 = np.ascontiguousarray(np.concatenate(
            [LTfull[h * R:(h + 1) * R], LTfull[(1 - h) * R:(2 - h) * R]],
            axis=0)).astype(f16)              # own j-half first
        xb = x[b]
        xloc = np.concatenate([xb[h * R:(h + 1) * R],
                               xb[(1 - h) * R:(2 - h) * R]], axis=0)
        xhat = np.ascontiguousarray(
            xloc.reshape(NJC, 128, F).transpose(1, 0, 2)).reshape(128, NJC * F)
        xs = np.ascontiguousarray(
            xb[rows].reshape(4, S, F).transpose(0, 2, 1)).reshape(128, S)
        in_maps.append({
            "LT": LT16,
            "xhat": xhat.astype(f16),
            "xs": xs.astype(f32),
            "xs16": xs.astype(f16),
            "wrep": wrep,
            "ident": ident,
        })
    return in_maps


def kernel(x, L, W, theta):
    from concourse import bass_utils

    x = np.asarray(x, dtype=np.float32)
    L = np.asarray(L, dtype=np.float32)
    W = np.asarray(W, dtype=np.float32)
    theta = np.asarray(theta, dtype=np.float32)

    if "nc" not in _CACHE:
        _CACHE["nc"] = _build()
    nc = _CACHE["nc"]

    in_maps = _host_prep(x, L, W, theta)
    trace = os.environ.get("BASS_CHEB_TRACE", "0") == "1"
    res = bass_utils.run_bass_kernel_spmd(
        nc, in_maps, core_ids=list(range(NCORES)), trace=trace)
    if trace:
        _CACHE["exec_time_ns"] = res.exec_time_ns
        _CACHE["trace"] = res.instructions_and_trace
        _CACHE["profile_json"] = res.profile_json
        print(f"HW exec time: {res.exec_time_ns} ns")

    out = np.empty((B, N, O), dtype=np.float32)
    for c in range(NCORES):
        b, h = c // 2, c % 2
        out[b, h * R:(h + 1) * R, :] = res.results[c]["y"].T
    return out

